# revision 1
# baseline (speedup 1.0000x reference)
"""Bass/Trainium2 kernel for nn_LogRatio loss, data-parallel over anchor rows on 8 cores.

Math: loss = sum_{m,j,k,l} pos[j,k] * N_m[j,l] * (A[j,k] - A[j,l] - c_m)^2
with A = log(X @ X.T + eps). All masks depend on labels only through the
anchor's class t_j (24 classes), so every masked row-reduction becomes a
matmul against a per-class 0/1 table W[l, c], followed by a per-row
selection of column c_j. Per core: 256 anchor rows; sim chunks are computed
transposed ([l partitions, j free]) so the same chunk feeds both the log
similarity and the class-table matmuls without any transposes.
"""

import numpy as np
import ml_dtypes

N, D, KK, C = 2048, 128, 4, 24
NCORES = 8
JPC = N // NCORES  # 256 anchor rows per core
NBLK = JPC // 128  # 2 blocks of 128 rows
NCH = N // 128     # 16 l-chunks
EPS = 1e-6
OMEGA = 0.1

_cache: dict = {}


def _build(repeats: int):
    import concourse.bacc as bacc
    import concourse.mybir as mybir
    import concourse.tile as tile

    f32 = mybir.dt.float32
    AL = mybir.AluOpType
    AF = mybir.ActivationFunctionType
    AX = mybir.AxisListType

    bf16 = mybir.dt.bfloat16
    nc = bacc.Bacc("TRN2", target_bir_lowering=False, debug=False)
    xt_d = nc.dram_tensor("xt", [D, N], bf16, kind="ExternalInput")
    xjt_d = nc.dram_tensor("xjt", [D, JPC], bf16, kind="ExternalInput")
    wt_d = nc.dram_tensor("wt", [NCH, 128, 72], f32, kind="ExternalInput")
    aux2_d = nc.dram_tensor("aux2", [128, 34], f32, kind="ExternalInput")
    loss_d = nc.dram_tensor("loss", [1, 1], f32, kind="ExternalOutput")

    with tile.TileContext(nc) as tc:
        with (
            tc.tile_pool(name="const", bufs=1) as const,
            tc.tile_pool(name="work", bufs=3) as work,
            tc.tile_pool(name="ps", bufs=1, space="PSUM") as ps,
            tc.tile_pool(name="psim", bufs=2, space="PSUM") as psim,
        ):
            def body():
                xt = const.tile([D, N], bf16, tag="xt")
                nc.sync.dma_start(xt[:], xt_d[:])
                xjt = const.tile([D, JPC], bf16, tag="xjt")
                nc.sync.dma_start(xjt[:], xjt_d[:])
                wt = const.tile([128, NCH, 72], f32, tag="wt")
                for i in range(NCH):
                    nc.sync.dma_start(wt[:, i, :], wt_d[i, :, :])
                aux2 = const.tile([128, 34], f32, tag="aux2")
                nc.sync.dma_start(aux2[:], aux2_d[:])
                tgt = aux2[:, 0:2]
                aux = aux2[:, 2:10].rearrange("p (b k) -> p b k", k=4)
                iota = aux2[:, 10:34]
                ones = const.tile([128, 1], f32, tag="ones")
                nc.vector.memset(ones[:], 1.0)
                epsb = const.tile([128, 1], f32, tag="epsb")
                nc.vector.memset(epsb[:], EPS)

                g1 = [ps.tile([128, 72], f32, tag=f"g1_{b}", name=f"g1_{b}") for b in range(NBLK)]
                g2 = [ps.tile([128, 48], f32, tag=f"g2_{b}", name=f"g2_{b}") for b in range(NBLK)]

                for i in range(NCH):
                    sim = psim.tile([128, JPC], f32, tag="sim")
                    nc.tensor.matmul(sim[:], xt[:, i * 128:(i + 1) * 128], xjt[:],
                                     start=True, stop=True)
                    a = work.tile([128, JPC], f32, tag="a")
                    nc.scalar.activation(a[:], sim[:], AF.Ln, bias=epsb[:])
                    a2 = work.tile([128, JPC], f32, tag="a2")
                    nc.vector.tensor_mul(a2[:], a[:], a[:])
                    for b in range(NBLK):
                        nc.tensor.matmul(g1[b][:], a[:, b * 128:(b + 1) * 128],
                                         wt[:, i, :], start=(i == 0), stop=(i == NCH - 1))
                        nc.tensor.matmul(g2[b][:], a2[:, b * 128:(b + 1) * 128],
                                         wt[:, i, 0:48], start=(i == 0), stop=(i == NCH - 1))

                # diagonal terms: dA_j = ln(||x_j||^2 + eps)
                sq = const.tile([D, JPC], f32, tag="sq")
                nc.vector.tensor_mul(sq[:], xjt[:], xjt[:])
                nrm = ps.tile([128, NBLK], f32, tag="nrm")
                for b in range(NBLK):
                    nc.tensor.matmul(nrm[:, b:b + 1], sq[:, b * 128:(b + 1) * 128],
                                     ones[:], start=True, stop=True)
                dA = const.tile([128, NBLK], f32, tag="dA")
                nc.scalar.activation(dA[:], nrm[:], AF.Ln, bias=epsb[:])
                dA2 = const.tile([128, NBLK], f32, tag="dA2")
                nc.vector.tensor_mul(dA2[:], dA[:], dA[:])

                # per-row selection of class column c_j from the G tables
                onehot = const.tile([128, NBLK, C], f32, tag="onehot")
                for b in range(NBLK):
                    nc.vector.tensor_scalar(onehot[:, b, :], iota, tgt[:, b:b + 1],
                                            None, AL.is_equal)
                gall = const.tile([128, NBLK, 120], f32, tag="gall")
                for b in range(NBLK):
                    nc.scalar.copy(gall[:, b, 0:72], g1[b][:])
                    nc.scalar.copy(gall[:, b, 72:120], g2[b][:])
                s1g = const.tile([128, NBLK], f32, tag="s1g")
                t1s = const.tile([128, NBLK], f32, tag="t1s")
                t1c = const.tile([128, NBLK], f32, tag="t1c")
                s2g = const.tile([128, NBLK], f32, tag="s2g")
                t2s = const.tile([128, NBLK], f32, tag="t2s")
                sels = [(0, s1g), (24, t1s), (48, t1c), (72, s2g), (96, t2s)]
                for off, dst in sels:
                    scr = work.tile([128, NBLK, C], f32, tag="scr")
                    nc.vector.tensor_mul(scr[:], gall[:, :, off:off + C], onehot[:])
                    nc.vector.tensor_reduce(dst[:], scr[:], axis=AX.X, op=AL.add)

                # per-row combine:
                # L = NnS*S2 - 2*S1*(NnC + T1S) + Pn*(NnC2 + 2*T1C + T2S)
                pn = aux[:, :, 0]
                nns = aux[:, :, 1]
                nnc = aux[:, :, 2]
                nnc2 = aux[:, :, 3]
                s1 = const.tile([128, NBLK], f32, tag="s1")
                nc.vector.tensor_sub(s1[:], s1g[:], dA[:])
                s2 = const.tile([128, NBLK], f32, tag="s2")
                nc.vector.tensor_sub(s2[:], s2g[:], dA2[:])
                u = const.tile([128, NBLK], f32, tag="u")
                nc.vector.tensor_add(u[:], nnc, t1s[:])
                v = const.tile([128, NBLK], f32, tag="v")
                nc.vector.tensor_mul(v[:], s1[:], u[:])
                w = const.tile([128, NBLK], f32, tag="w")
                nc.vector.tensor_scalar(w[:], t1c[:], 2.0, None, AL.mult)
                nc.vector.tensor_add(w[:], w[:], t2s[:])
                nc.vector.tensor_add(w[:], w[:], nnc2)
                xx = const.tile([128, NBLK], f32, tag="xx")
                nc.vector.tensor_mul(xx[:], pn, w[:])
                ll = const.tile([128, NBLK], f32, tag="ll")
                nc.vector.tensor_mul(ll[:], nns, s2[:])
                nc.vector.tensor_add(ll[:], ll[:], xx[:])
                nc.vector.tensor_sub(ll[:], ll[:], v[:])
                nc.vector.tensor_sub(ll[:], ll[:], v[:])
                lp = const.tile([128, 1], f32, tag="lp")
                nc.vector.tensor_reduce(lp[:], ll[:], axis=AX.X, op=AL.add)
                lps = ps.tile([1, 1], f32, tag="loss")
                nc.tensor.matmul(lps[:], lp[:], ones[:], start=True, stop=True)
                lsb = const.tile([1, 1], f32, tag="lsb")
                nc.scalar.copy(lsb[:], lps[:])
                nc.sync.dma_start(loss_d[:], lsb[:])

            if repeats == 1:
                body()
            else:
                with tc.For_i(0, repeats, 1):
                    body()

    nc.compile()
    return nc


def _prep_inputs(inputs: np.ndarray, labels: np.ndarray):
    X = np.asarray(inputs, dtype=np.float32)
    lab = np.asarray(labels).astype(np.int64)
    XT = np.ascontiguousarray(X.T)  # [128, 2048]
    t = lab[:, 0]

    E = (lab[:, :, None] == np.arange(C)[None, None, :]).astype(np.float32)  # [N, 4, C]
    Wpos = E[:, 0]
    W0 = 1.0 - E[:, 3]
    W1 = E[:, 3] * (1.0 - E[:, 2])
    W2 = E[:, 2] * (1.0 - E[:, 1])
    W3 = E[:, 1] * (1.0 - E[:, 0])
    cm = np.array(
        [0.1 * (np.log(OMEGA + EPS) - np.log(OMEGA ** (KK - m + 1) + EPS)) for m in range(KK)],
        dtype=np.float32,
    )
    Wsum = W0 + W1 + W2 + W3
    Wc = cm[0] * W0 + cm[1] * W1 + cm[2] * W2 + cm[3] * W3
    Wtbl = np.concatenate([Wpos, Wsum, Wc], axis=1).astype(np.float32)  # [N, 72]
    Wtbl = np.ascontiguousarray(Wtbl.reshape(NCH, 128, 72))

    colsum = np.stack([W.sum(axis=0) for W in (W0, W1, W2, W3)])  # [4, C]
    cnt0 = Wpos.sum(axis=0)  # [C]
    NnS_c = colsum.sum(axis=0)
    NnC_c = (cm[:, None] * colsum).sum(axis=0)
    NnC2_c = ((cm ** 2)[:, None] * colsum).sum(axis=0)
    aux = np.stack(
        [cnt0[t] - 1.0, NnS_c[t], NnC_c[t], NnC2_c[t]], axis=1
    ).astype(np.float32)  # [N, 4]
    tgtf = t.astype(np.float32)
    iota = np.ascontiguousarray(
        np.broadcast_to(np.arange(C, dtype=np.float32), (128, C))
    )

    XTb = XT.astype(ml_dtypes.bfloat16)
    in_maps = []
    for core in range(NCORES):
        j0 = core * JPC
        aux2 = np.zeros((128, 34), dtype=np.float32)
        aux2[:, 0:2] = tgtf[j0:j0 + JPC].reshape(NBLK, 128).T
        aux2[:, 2:10] = aux[j0:j0 + JPC].reshape(NBLK, 128, 4).transpose(1, 0, 2).reshape(128, 8)
        aux2[:, 10:34] = iota
        in_maps.append({
            "xt": XTb,
            "xjt": np.ascontiguousarray(XTb[:, j0:j0 + JPC]),
            "wt": Wtbl,
            "aux2": aux2,
        })
    return in_maps


def _get_nc(repeats: int = 1):
    key = ("nc", repeats)
    if key not in _cache:
        _cache[key] = _build(repeats)
    return _cache[key]


def run_on_device(inputs, labels, repeats: int = 1):
    from concourse.bass_utils import run_bass_kernel_spmd

    nc = _get_nc(repeats)
    in_maps = _prep_inputs(inputs, labels)
    res = run_bass_kernel_spmd(nc, in_maps, list(range(NCORES)))
    total = np.float64(0.0)
    partials = [res.results[i]["loss"][0, 0] for i in range(NCORES)]
    total = np.float32(np.sum(np.asarray(partials, dtype=np.float32)))
    return total, partials


def kernel(inputs, labels):
    total, _ = run_on_device(inputs, labels, repeats=1)
    return (total, 0, 0, 0)

